# revision 16
# baseline (speedup 1.0000x reference)
"""Per-pixel adaptive 5x5 conv (KPN) for Trainium2, 8-core data parallel.

out[g,h,w] = sum_{i,j} core[g,5i+j,h,w] * frames_pad[g,h+i-2,w+j-2]
with g = flattened (B,N) = 16 image planes; 2 planes per NeuronCore,
fused into one free dim so every elementwise op covers both.

v2 layout (vs v1's parity-copy scheme): each 128-row block stores 516
frame cols (out cols plus the +-2 halo), so ONE frame tile per row
shift i serves all 5 column taps j of its group: the column shift is
folded into the host weight layout (w'[c'] = w[c'-j]) and the PE reads
each product tile at free-dim offset j when accumulating bank b over
cols [b*516+j, b*516+j+512).  Frame DMA drops from 10.6 to 5.3 MB/core.

Engine split:
  DVE   - 25 products w_t*f_t (fp16 2x mode, ~2.2us each) plus the
          t=0 weight dequant (rides the ramp shadow)
  ACT   - 15 weight dequants (int8 codes -> fp16 copy, 3.6us each)
  DMA   - 9 weight tiles stored as fp16 codes in DRAM and loaded
          directly (no dequant anywhere; costs +0.53 MB of DMA each,
          cheaper than an engine cast while the queues have slack;
          SWDGE casting DMA was measured to cost read+write on the
          queues, worse than both)
  PE    - accumulates the 25 product streams into PSUM fp32 via
          matmuls against a stationary (2^-5 * I); 8 banks = [128,512]
  tail  - PSUM evac split scalar/vector engines, output stored in two
          0.5 MB chunks

Group 0 is split into per-image half tiles ([128,2064]) so the first
dequant+mul start as soon as ~0.25 MB lands instead of waiting for the
full 1.6 MB head (the DMA queues round-robin all in-flight transfers,
so first-tile latency is proportional to bytes in flight).

Weights are int8 (w8 = clip(round(w * 32), -127, 127)); the 2^-5
dequant scale is folded into the PE's stationary identity, so every
dequant path (ACT copy, DVE copy, casting DMA) is a pure cast.

Host layouts:
  fin [5, 128, 4128] fp16: fin[i][p, (img,blk,c)] =
     Fpad[img, blk*128+p+i, c], Fpad = pad(F, rows 2/2, cols 2/2),
     c in [0,516).
  win [25, 128, 4128] int8: win[t][p, (img,blk,c')] =
     clip(round(32*core[img, t, blk*128+p, c'-j]), -127, 127) for
     c'-j in [0,512) else 0, where j = t%5.
  oout [128, 4096] fp16 (host casts to f32).
"""

import os
import sys

import numpy as np

for _p in ("/opt/trn_rl_repo",):
    if _p not in sys.path and os.path.isdir(_p):
        sys.path.insert(0, _p)

K = 5
NCORES = 8
IMGS_PER_CORE = 2
H = W = 512
NBLK = 4          # 128-row blocks per image
C_BLK = 516       # 512 out cols + 4 halo cols (-2..513)
FREE = IMGS_PER_CORE * NBLK * C_BLK   # 4128
HFREE = FREE // 2                     # 2064 = one image
O_FREE = IMGS_PER_CORE * NBLK * W     # 4096
NBANK = 8
BANK = 512
WSCALE = 2.0 ** -5  # int8 weight dequant scale, folded into PE identity

# Dequant engine assignment per tap t = 5*i + j.
DVE_DEQ = frozenset()
FP16_W = (0, 1, 4, 8, 9, 13, 14, 18, 19, 23, 24)  # stored fp16, no dequant
FP16_IDX = {t: n for n, t in enumerate(FP16_W)}
# remaining 15 taps dequant on ACT

_compiled = {}
last_results = None  # BassKernelResults of the most recent run (for test.py)


def _build_nc():
    import concourse.bacc as bacc
    import concourse.mybir as mybir
    from concourse.tile import TileContext

    f16 = mybir.dt.float16
    f32 = mybir.dt.float32
    i8 = mybir.dt.int8

    nc = bacc.Bacc(None, target_bir_lowering=False, debug=False)
    ident = nc.dram_tensor("ident", [128, 128], f16, kind="ExternalInput")
    fin = nc.dram_tensor("fin", [K, 128, FREE], f16, kind="ExternalInput")
    win = nc.dram_tensor("win", [K * K, 128, FREE], i8, kind="ExternalInput")
    win16 = nc.dram_tensor("win16", [len(FP16_W), 128, FREE], f16,
                           kind="ExternalInput")
    oout = nc.dram_tensor("oout", [128, O_FREE], f16, kind="ExternalOutput")

    n_streams = K * K

    with TileContext(nc) as tc:
        with (
            tc.tile_pool(name="ipool", bufs=1) as ipool,
            tc.tile_pool(name="fpool", bufs=3) as fpool,
            tc.tile_pool(name="fhpool", bufs=1) as fhpool,
            tc.tile_pool(name="w8pool", bufs=2) as w8pool,
            tc.tile_pool(name="whpool", bufs=1) as whpool,
            tc.tile_pool(name="wpool", bufs=2) as wpool,
            tc.tile_pool(name="spool", bufs=3) as spool,
            tc.tile_pool(name="shpool", bufs=3) as shpool,
            tc.tile_pool(name="opool", bufs=1) as opool,
            tc.tile_pool(name="ppool", bufs=1, space="PSUM") as ppool,
        ):
            id_t = ipool.tile([128, 128], f16, tag="ident")

            banks = [ppool.tile([128, BANK], f32, tag=f"b{b}",
                                name=f"bank{b}")
                     for b in range(NBANK)]
            osb = opool.tile([128, O_FREE], f16, tag="osb")

            f_tiles = {}
            w8_tiles = {}
            w_tiles = {}
            bank_n = [0] * NBANK

            def pe_acc(tile, j, bank_list, off0):
                # rhs covers out cols of bank b at product offset j
                for lb, b in enumerate(bank_list):
                    s = bank_n[b]
                    bank_n[b] += 1
                    nc.tensor.matmul(
                        out=banks[b][:],
                        lhsT=id_t[:],
                        rhs=tile[:][:, off0 + lb * C_BLK + j:
                                    off0 + lb * C_BLK + j + BANK],
                        start=(s == 0),
                        stop=(s == n_streams - 1),
                    )

            def emit_w(tg, k):
                t = tg * K + k
                if t in FP16_IDX:
                    # fp16 codes straight from DRAM, no dequant step
                    w_t = wpool.tile([128, FREE], f16, tag=f"w{k}",
                                     name=f"wf16_{t}")
                    nc.sync.dma_start(out=w_t[:], in_=win16[FP16_IDX[t]])
                    w_tiles[t] = w_t
                    return
                w8_t = w8pool.tile([128, FREE], i8, tag=f"w8{k}",
                                   name=f"w8_{t}")
                nc.sync.dma_start(out=w8_t[:], in_=win[t])
                w8_tiles[t] = w8_t
                # ACT dequants are emitted here so ACT chases the DMA
                # arrivals a group ahead of the DVE muls; DVE's own
                # dequants are emitted inline in emit_compute.
                if t not in DVE_DEQ:
                    w_t = wpool.tile([128, FREE], f16, tag=f"w{k}",
                                     name=f"wdq{t}")
                    nc.scalar.copy(out=w_t[:], in_=w8_t[:])
                    w_tiles[t] = w_t

            def emit_compute(tg):
                for k in range(K):
                    t = tg * K + k
                    if t in DVE_DEQ:
                        w_t = wpool.tile([128, FREE], f16, tag=f"w{k}",
                                         name=f"wdq{t}")
                        nc.vector.tensor_copy(out=w_t[:],
                                              in_=w8_tiles[t][:])
                        w_tiles[t] = w_t
                    if t == n_streams - 1:
                        # final tap in halves so the PSUM evac + store
                        # tail starts ~1us earlier
                        for h in range(2):
                            sl = slice(h * HFREE, (h + 1) * HFREE)
                            tmp = shpool.tile([128, HFREE], f16, tag="sh")
                            nc.vector.tensor_mul(
                                out=tmp[:], in0=w_tiles[t][:][:, sl],
                                in1=f_tiles[tg][:][:, sl])
                            pe_acc(tmp, k, range(4 * h, 4 * h + 4), 0)
                        continue
                    tmp = spool.tile([128, FREE], f16, tag="s")
                    nc.vector.tensor_mul(out=tmp[:], in0=w_tiles[t][:],
                                         in1=f_tiles[tg][:])
                    pe_acc(tmp, k, range(NBANK), 0)

            def emit_f(tg):
                f_t = fpool.tile([128, FREE], f16, tag="f",
                                 name=f"fr{tg}")
                nc.sync.dma_start(out=f_t[:], in_=fin[tg])
                f_tiles[tg] = f_t

            # ---- head: group 0 in per-image halves for a fast ramp;
            # t0 is fp16-direct so the first mul is DMA-gated only ----
            wh16 = []
            fh = []
            for h in range(2):
                wh16.append(whpool.tile([128, HFREE], f16, tag=f"wh{h}",
                                        name=f"wh16_{h}"))
                fh.append(fhpool.tile([128, HFREE], f16, tag=f"fh{h}",
                                      name=f"fh{h}"))
            # head tiles split across the two HWDGE rings (sync + scalar)
            # so the first mul's pair shares queue service two ways, not
            # six: each SDMA engine round-robins rows at packet
            # granularity, so fewer concurrent transfers = faster first
            # arrival.
            i0 = FP16_IDX[0]
            nc.sync.dma_start(out=wh16[0][:], in_=win16[i0][:, :HFREE])
            nc.scalar.dma_start(out=fh[0][:], in_=fin[0][:, :HFREE])
            nc.scalar.dma_start(out=wh16[1][:], in_=win16[i0][:, HFREE:])
            nc.sync.dma_start(out=fh[1][:], in_=fin[0][:, HFREE:])

            # DMAs globally ordered by consumption deadline: ACT-cast
            # (int8) taps get issued ~a group ahead of the fp16-direct
            # taps, since their arrival is followed by a 3.6us serial
            # cast on ACT before the DVE mul can consume them.
            emit_w(0, 2)
            emit_w(0, 3)
            emit_w(0, 1)
            emit_w(1, 0)
            emit_w(1, 1)
            emit_w(0, 4)
            emit_w(1, 2)
            emit_f(1)
            # identity rides the (empty) ACT hwdge ring, keeping its 128
            # tiny descriptors out of the sync ring's ramp window
            nc.scalar.dma_start(out=id_t[:], in_=ident[:])

            # group 0 compute: 10 half-muls, no casts anywhere
            for k in range(K):
                for h in range(2):
                    if k == 0:
                        w_ap = wh16[h][:]
                    else:
                        w_ap = w_tiles[k][:][:, h * HFREE:(h + 1) * HFREE]
                    tmp = shpool.tile([128, HFREE], f16, tag="sh")
                    nc.vector.tensor_mul(out=tmp[:], in0=w_ap,
                                         in1=fh[h][:])
                    pe_acc(tmp, k, range(4 * h, 4 * h + 4), 0)

            emit_w(2, 0)
            emit_w(1, 3)
            emit_w(2, 1)
            emit_w(1, 4)
            emit_w(2, 2)
            emit_f(2)
            emit_compute(1)
            emit_w(3, 0)
            emit_w(2, 3)
            emit_w(3, 1)
            emit_w(2, 4)
            emit_w(3, 2)
            emit_f(3)
            emit_compute(2)
            emit_w(4, 0)
            emit_w(3, 3)
            emit_w(4, 1)
            emit_w(3, 4)
            emit_w(4, 2)
            emit_f(4)
            emit_compute(3)
            emit_w(4, 3)
            emit_w(4, 4)
            emit_compute(K - 1)

            # ---- tail: PSUM fp32 -> SBUF fp16 split across scalar and
            # vector engines, store output in two chunks ----
            for b in range(NBANK):
                dst = osb[:][:, b * BANK:(b + 1) * BANK]
                if b % 2 == 0:
                    nc.scalar.copy(out=dst, in_=banks[b][:])
                else:
                    nc.vector.tensor_copy(out=dst, in_=banks[b][:])
                if b % 2 == 1:
                    c0 = (b - 1) * BANK
                    nc.sync.dma_start(out=oout[:, c0:c0 + 2 * BANK],
                                      in_=osb[:][:, c0:c0 + 2 * BANK])
    nc.finalize()
    return nc


def _host_prep(frames, core):
    """Build per-core in_maps. frames [4,4,1,512,512] f32, core [4,4,25,1,512,512]."""
    G = NCORES * IMGS_PER_CORE  # 16
    F = np.ascontiguousarray(frames.reshape(G, H, W))
    Wc = core.reshape(G, K * K, H, W)

    # frames: pad rows 2/2, cols 2/2 -> [G, 516, 516]
    Fp = np.pad(F, ((0, 0), (2, 2), (2, 2))).astype(np.float16)
    # A[g, i, blk, p, c] = Fp[g, blk*128+p+i, c]
    A = np.empty((G, K, NBLK, 128, C_BLK), np.float16)
    for i in range(K):
        A[:, i] = Fp[:, i:i + H, :].reshape(G, NBLK, 128, C_BLK)
    # fin[core][i, p, (img, blk, c)]
    fprep = np.ascontiguousarray(
        A.reshape(NCORES, IMGS_PER_CORE, K, NBLK, 128, C_BLK)
        .transpose(0, 2, 4, 1, 3, 5))

    # weights: codes of w/2^-5, column-shifted by j so products line up
    # with an aligned frame read; PE reads the product at offset j.
    # int8-rounded codes for the engine-cast taps, full fp16 codes for
    # the direct-load taps.
    w8 = np.clip(np.round(Wc * (1.0 / WSCALE)), -127, 127).astype(np.int8)
    Ws = np.zeros((G, K * K, H, C_BLK), np.int8)
    for j in range(K):
        Ws[:, j::K, :, j:j + W] = w8[:, j::K]
    wprep = np.ascontiguousarray(
        Ws.reshape(NCORES, IMGS_PER_CORE, K * K, NBLK, 128, C_BLK)
        .transpose(0, 2, 4, 1, 3, 5))

    tf = list(FP16_W)
    Wf = np.zeros((G, len(tf), H, C_BLK), np.float16)
    for n, t in enumerate(tf):
        j = t % K
        Wf[:, n, :, j:j + W] = (Wc[:, t] * (1.0 / WSCALE)).astype(np.float16)
    wfprep = np.ascontiguousarray(
        Wf.reshape(NCORES, IMGS_PER_CORE, len(tf), NBLK, 128, C_BLK)
        .transpose(0, 2, 4, 1, 3, 5))

    ident = (np.eye(128) * WSCALE).astype(np.float16)
    in_maps = []
    for c in range(NCORES):
        in_maps.append({
            "ident": ident,
            "fin": fprep[c].reshape(K, 128, FREE),
            "win": wprep[c].reshape(K * K, 128, FREE),
            "win16": wfprep[c].reshape(len(tf), 128, FREE),
        })
    return in_maps


def kernel(frames, core, bias):
    global last_results
    from concourse.bass_utils import run_bass_kernel_spmd

    frames = np.asarray(frames, dtype=np.float32)
    core = np.asarray(core, dtype=np.float32)

    if "nc" not in _compiled:
        _compiled["nc"] = _build_nc()
    nc = _compiled["nc"]

    in_maps = _host_prep(frames, core)
    trace = os.environ.get("KC_TRACE") == "1"
    tmpdir = os.environ.get("KC_TRACE_DIR") or None
    if tmpdir:
        os.makedirs(tmpdir, exist_ok=True)
    res = run_bass_kernel_spmd(nc, in_maps, list(range(NCORES)), trace=trace,
                               tmpdir=tmpdir)
    last_results = res

    G = NCORES * IMGS_PER_CORE
    out = np.empty((G, H, W), np.float32)
    for c in range(NCORES):
        o = res.results[c]["oout"]  # [128, 4096] f16
        ov = o.reshape(128, IMGS_PER_CORE, NBLK, W).astype(np.float32)
        for img in range(IMGS_PER_CORE):
            out[c * IMGS_PER_CORE + img] = (
                ov[:, img].transpose(1, 0, 2).reshape(H, W))
    return out.reshape(4, 4, H, W)


# revision 17
# speedup vs baseline: 1.0144x; 1.0144x over previous
"""Per-pixel adaptive 5x5 conv (KPN) for Trainium2, 8-core data parallel.

out[g,h,w] = sum_{i,j} core[g,5i+j,h,w] * frames_pad[g,h+i-2,w+j-2]
with g = flattened (B,N) = 16 image planes; 2 planes per NeuronCore,
fused into one free dim so every elementwise op covers both.

v2 layout (vs v1's parity-copy scheme): each 128-row block stores 516
frame cols (out cols plus the +-2 halo), so ONE frame tile per row
shift i serves all 5 column taps j of its group: the column shift is
folded into the host weight layout (w'[c'] = w[c'-j]) and the PE reads
each product tile at free-dim offset j when accumulating bank b over
cols [b*516+j, b*516+j+512).  Frame DMA drops from 10.6 to 5.3 MB/core.

Engine split:
  DVE   - 25 products w_t*f_t (fp16 2x mode, ~2.2us each) plus the
          t=0 weight dequant (rides the ramp shadow)
  ACT   - 15 weight dequants (int8 codes -> fp16 copy, 3.6us each)
  DMA   - 9 weight tiles stored as fp16 codes in DRAM and loaded
          directly (no dequant anywhere; costs +0.53 MB of DMA each,
          cheaper than an engine cast while the queues have slack;
          SWDGE casting DMA was measured to cost read+write on the
          queues, worse than both)
  PE    - accumulates the 25 product streams into PSUM fp32 via
          matmuls against a stationary (2^-5 * I); 8 banks = [128,512]
  tail  - PSUM evac split scalar/vector engines, output stored in two
          0.5 MB chunks

Group 0 is split into per-image half tiles ([128,2064]) so the first
dequant+mul start as soon as ~0.25 MB lands instead of waiting for the
full 1.6 MB head (the DMA queues round-robin all in-flight transfers,
so first-tile latency is proportional to bytes in flight).

Weights are int8 (w8 = clip(round(w * 32), -127, 127)); the 2^-5
dequant scale is folded into the PE's stationary identity, so every
dequant path (ACT copy, DVE copy, casting DMA) is a pure cast.

Host layouts:
  fin [5, 128, 4128] fp16: fin[i][p, (img,blk,c)] =
     Fpad[img, blk*128+p+i, c], Fpad = pad(F, rows 2/2, cols 2/2),
     c in [0,516).
  win [25, 128, 4128] int8: win[t][p, (img,blk,c')] =
     clip(round(32*core[img, t, blk*128+p, c'-j]), -127, 127) for
     c'-j in [0,512) else 0, where j = t%5.
  oout [128, 4096] fp16 (host casts to f32).
"""

import os
import sys

import numpy as np

for _p in ("/opt/trn_rl_repo",):
    if _p not in sys.path and os.path.isdir(_p):
        sys.path.insert(0, _p)

K = 5
NCORES = 8
IMGS_PER_CORE = 2
H = W = 512
NBLK = 4          # 128-row blocks per image
C_BLK = 516       # 512 out cols + 4 halo cols (-2..513)
FREE = IMGS_PER_CORE * NBLK * C_BLK   # 4128
HFREE = FREE // 2                     # 2064 = one image
O_FREE = IMGS_PER_CORE * NBLK * W     # 4096
NBANK = 8
BANK = 512
WSCALE = 2.0 ** -5  # int8 weight dequant scale, folded into PE identity

# Dequant engine assignment per tap t = 5*i + j.
DVE_DEQ = frozenset()
FP16_W = (0, 1, 4, 8, 9, 13, 14, 18, 19, 23, 24)  # stored fp16, no dequant
FP16_IDX = {t: n for n, t in enumerate(FP16_W)}
# remaining 15 taps dequant on ACT

_compiled = {}
last_results = None  # BassKernelResults of the most recent run (for test.py)


def _build_nc():
    import concourse.bacc as bacc
    import concourse.mybir as mybir
    from concourse.tile import TileContext

    f16 = mybir.dt.float16
    f32 = mybir.dt.float32
    i8 = mybir.dt.int8

    nc = bacc.Bacc(None, target_bir_lowering=False, debug=False)
    ident = nc.dram_tensor("ident", [128, 128], f16, kind="ExternalInput")
    fin = nc.dram_tensor("fin", [K, 128, FREE], f16, kind="ExternalInput")
    win = nc.dram_tensor("win", [K * K, 128, FREE], i8, kind="ExternalInput")
    win16 = nc.dram_tensor("win16", [len(FP16_W), 128, FREE], f16,
                           kind="ExternalInput")
    oout = nc.dram_tensor("oout", [128, O_FREE], f16, kind="ExternalOutput")

    n_streams = K * K

    with TileContext(nc) as tc:
        with (
            tc.tile_pool(name="ipool", bufs=1) as ipool,
            tc.tile_pool(name="fpool", bufs=3) as fpool,
            tc.tile_pool(name="fhpool", bufs=1) as fhpool,
            tc.tile_pool(name="w8pool", bufs=2) as w8pool,
            tc.tile_pool(name="whpool", bufs=1) as whpool,
            tc.tile_pool(name="wpool", bufs=2) as wpool,
            tc.tile_pool(name="spool", bufs=3) as spool,
            tc.tile_pool(name="shpool", bufs=3) as shpool,
            tc.tile_pool(name="opool", bufs=1) as opool,
            tc.tile_pool(name="ppool", bufs=1, space="PSUM") as ppool,
        ):
            id_t = ipool.tile([128, 128], f16, tag="ident")

            banks = [ppool.tile([128, BANK], f32, tag=f"b{b}",
                                name=f"bank{b}")
                     for b in range(NBANK)]
            osb = opool.tile([128, O_FREE], f16, tag="osb")

            f_tiles = {}
            w8_tiles = {}
            w_tiles = {}
            bank_n = [0] * NBANK

            def pe_acc(tile, j, bank_list, off0):
                # rhs covers out cols of bank b at product offset j
                for lb, b in enumerate(bank_list):
                    s = bank_n[b]
                    bank_n[b] += 1
                    nc.tensor.matmul(
                        out=banks[b][:],
                        lhsT=id_t[:],
                        rhs=tile[:][:, off0 + lb * C_BLK + j:
                                    off0 + lb * C_BLK + j + BANK],
                        start=(s == 0),
                        stop=(s == n_streams - 1),
                    )

            def emit_w(tg, k):
                t = tg * K + k
                if t in FP16_IDX:
                    # fp16 codes straight from DRAM, no dequant step
                    w_t = wpool.tile([128, FREE], f16, tag=f"w{k}",
                                     name=f"wf16_{t}")
                    nc.sync.dma_start(out=w_t[:], in_=win16[FP16_IDX[t]])
                    w_tiles[t] = w_t
                    return
                w8_t = w8pool.tile([128, FREE], i8, tag=f"w8{k}",
                                   name=f"w8_{t}")
                nc.sync.dma_start(out=w8_t[:], in_=win[t])
                w8_tiles[t] = w8_t
                # ACT dequants are emitted here so ACT chases the DMA
                # arrivals a group ahead of the DVE muls; DVE's own
                # dequants are emitted inline in emit_compute.
                if t not in DVE_DEQ:
                    w_t = wpool.tile([128, FREE], f16, tag=f"w{k}",
                                     name=f"wdq{t}")
                    nc.scalar.copy(out=w_t[:], in_=w8_t[:])
                    w_tiles[t] = w_t

            def emit_compute(tg):
                for k in range(K):
                    t = tg * K + k
                    if t in DVE_DEQ:
                        w_t = wpool.tile([128, FREE], f16, tag=f"w{k}",
                                         name=f"wdq{t}")
                        nc.vector.tensor_copy(out=w_t[:],
                                              in_=w8_tiles[t][:])
                        w_tiles[t] = w_t
                    if t == n_streams - 1:
                        # final tap in halves so the PSUM evac + store
                        # tail starts ~1us earlier
                        for h in range(2):
                            sl = slice(h * HFREE, (h + 1) * HFREE)
                            tmp = shpool.tile([128, HFREE], f16, tag="sh")
                            nc.vector.tensor_mul(
                                out=tmp[:], in0=w_tiles[t][:][:, sl],
                                in1=f_tiles[tg][:][:, sl])
                            pe_acc(tmp, k, range(4 * h, 4 * h + 4), 0)
                        continue
                    tmp = spool.tile([128, FREE], f16, tag="s")
                    nc.vector.tensor_mul(out=tmp[:], in0=w_tiles[t][:],
                                         in1=f_tiles[tg][:])
                    pe_acc(tmp, k, range(NBANK), 0)

            def emit_f(tg):
                f_t = fpool.tile([128, FREE], f16, tag="f",
                                 name=f"fr{tg}")
                nc.sync.dma_start(out=f_t[:], in_=fin[tg])
                f_tiles[tg] = f_t

            # ---- head: group 0 in per-image halves for a fast ramp;
            # t0 is fp16-direct so the first mul is DMA-gated only ----
            wh16 = []
            fh = []
            for h in range(2):
                wh16.append(whpool.tile([128, HFREE], f16, tag=f"wh{h}",
                                        name=f"wh16_{h}"))
                fh.append(fhpool.tile([128, HFREE], f16, tag=f"fh{h}",
                                      name=f"fh{h}"))
            i0 = FP16_IDX[0]
            nc.sync.dma_start(out=wh16[0][:], in_=win16[i0][:, :HFREE])
            nc.sync.dma_start(out=fh[0][:], in_=fin[0][:, :HFREE])
            nc.sync.dma_start(out=wh16[1][:], in_=win16[i0][:, HFREE:])
            nc.sync.dma_start(out=fh[1][:], in_=fin[0][:, HFREE:])

            # DMAs globally ordered by consumption deadline: ACT-cast
            # (int8) taps get issued ~a group ahead of the fp16-direct
            # taps, since their arrival is followed by a 3.6us serial
            # cast on ACT before the DVE mul can consume them.
            emit_w(0, 2)
            emit_w(0, 3)
            emit_w(0, 1)
            emit_w(1, 0)
            emit_w(1, 1)
            emit_w(0, 4)
            emit_w(1, 2)
            emit_f(1)
            # identity rides the (empty) ACT hwdge ring, keeping its 128
            # tiny descriptors out of the sync ring's ramp window
            nc.scalar.dma_start(out=id_t[:], in_=ident[:])

            # group 0 compute: 10 half-muls, no casts anywhere
            for k in range(K):
                for h in range(2):
                    if k == 0:
                        w_ap = wh16[h][:]
                    else:
                        w_ap = w_tiles[k][:][:, h * HFREE:(h + 1) * HFREE]
                    tmp = shpool.tile([128, HFREE], f16, tag="sh")
                    nc.vector.tensor_mul(out=tmp[:], in0=w_ap,
                                         in1=fh[h][:])
                    pe_acc(tmp, k, range(4 * h, 4 * h + 4), 0)

            emit_w(2, 0)
            emit_w(1, 3)
            emit_w(2, 1)
            emit_w(1, 4)
            emit_w(2, 2)
            emit_f(2)
            emit_compute(1)
            emit_w(3, 0)
            emit_w(2, 3)
            emit_w(3, 1)
            emit_w(2, 4)
            emit_w(3, 2)
            emit_f(3)
            emit_compute(2)
            emit_w(4, 0)
            emit_w(3, 3)
            emit_w(4, 1)
            emit_w(3, 4)
            emit_w(4, 2)
            emit_f(4)
            emit_compute(3)
            emit_w(4, 3)
            emit_w(4, 4)
            emit_compute(K - 1)

            # ---- tail: PSUM fp32 -> SBUF fp16 split across scalar and
            # vector engines, store output in two chunks ----
            for b in range(NBANK):
                dst = osb[:][:, b * BANK:(b + 1) * BANK]
                if b % 2 == 0:
                    nc.scalar.copy(out=dst, in_=banks[b][:])
                else:
                    nc.vector.tensor_copy(out=dst, in_=banks[b][:])
                if b % 2 == 1:
                    c0 = (b - 1) * BANK
                    nc.sync.dma_start(out=oout[:, c0:c0 + 2 * BANK],
                                      in_=osb[:][:, c0:c0 + 2 * BANK])
    nc.finalize()
    return nc


def _host_prep(frames, core):
    """Build per-core in_maps. frames [4,4,1,512,512] f32, core [4,4,25,1,512,512]."""
    G = NCORES * IMGS_PER_CORE  # 16
    F = np.ascontiguousarray(frames.reshape(G, H, W))
    Wc = core.reshape(G, K * K, H, W)

    # frames: pad rows 2/2, cols 2/2 -> [G, 516, 516]
    Fp = np.pad(F, ((0, 0), (2, 2), (2, 2))).astype(np.float16)
    # A[g, i, blk, p, c] = Fp[g, blk*128+p+i, c]
    A = np.empty((G, K, NBLK, 128, C_BLK), np.float16)
    for i in range(K):
        A[:, i] = Fp[:, i:i + H, :].reshape(G, NBLK, 128, C_BLK)
    # fin[core][i, p, (img, blk, c)]
    fprep = np.ascontiguousarray(
        A.reshape(NCORES, IMGS_PER_CORE, K, NBLK, 128, C_BLK)
        .transpose(0, 2, 4, 1, 3, 5))

    # weights: codes of w/2^-5, column-shifted by j so products line up
    # with an aligned frame read; PE reads the product at offset j.
    # int8-rounded codes for the engine-cast taps, full fp16 codes for
    # the direct-load taps.
    w8 = np.clip(np.round(Wc * (1.0 / WSCALE)), -127, 127).astype(np.int8)
    Ws = np.zeros((G, K * K, H, C_BLK), np.int8)
    for j in range(K):
        Ws[:, j::K, :, j:j + W] = w8[:, j::K]
    wprep = np.ascontiguousarray(
        Ws.reshape(NCORES, IMGS_PER_CORE, K * K, NBLK, 128, C_BLK)
        .transpose(0, 2, 4, 1, 3, 5))

    tf = list(FP16_W)
    Wf = np.zeros((G, len(tf), H, C_BLK), np.float16)
    for n, t in enumerate(tf):
        j = t % K
        Wf[:, n, :, j:j + W] = (Wc[:, t] * (1.0 / WSCALE)).astype(np.float16)
    wfprep = np.ascontiguousarray(
        Wf.reshape(NCORES, IMGS_PER_CORE, len(tf), NBLK, 128, C_BLK)
        .transpose(0, 2, 4, 1, 3, 5))

    ident = (np.eye(128) * WSCALE).astype(np.float16)
    in_maps = []
    for c in range(NCORES):
        in_maps.append({
            "ident": ident,
            "fin": fprep[c].reshape(K, 128, FREE),
            "win": wprep[c].reshape(K * K, 128, FREE),
            "win16": wfprep[c].reshape(len(tf), 128, FREE),
        })
    return in_maps


def kernel(frames, core, bias):
    global last_results
    from concourse.bass_utils import run_bass_kernel_spmd

    frames = np.asarray(frames, dtype=np.float32)
    core = np.asarray(core, dtype=np.float32)

    if "nc" not in _compiled:
        _compiled["nc"] = _build_nc()
    nc = _compiled["nc"]

    in_maps = _host_prep(frames, core)
    trace = os.environ.get("KC_TRACE") == "1"
    tmpdir = os.environ.get("KC_TRACE_DIR") or None
    if tmpdir:
        os.makedirs(tmpdir, exist_ok=True)
    res = run_bass_kernel_spmd(nc, in_maps, list(range(NCORES)), trace=trace,
                               tmpdir=tmpdir)
    last_results = res

    G = NCORES * IMGS_PER_CORE
    out = np.empty((G, H, W), np.float32)
    for c in range(NCORES):
        o = res.results[c]["oout"]  # [128, 4096] f16
        ov = o.reshape(128, IMGS_PER_CORE, NBLK, W).astype(np.float32)
        for img in range(IMGS_PER_CORE):
            out[c * IMGS_PER_CORE + img] = (
                ov[:, img].transpose(1, 0, 2).reshape(H, W))
    return out.reshape(4, 4, H, W)


# revision 19
# speedup vs baseline: 1.0689x; 1.0537x over previous
"""Per-pixel adaptive 5x5 conv (KPN) for Trainium2, 8-core data parallel.

out[g,h,w] = sum_{i,j} core[g,5i+j,h,w] * frames_pad[g,h+i-2,w+j-2]
with g = flattened (B,N) = 16 image planes; 2 planes per NeuronCore,
fused into one free dim so every elementwise op covers both.

v2 layout (vs v1's parity-copy scheme): each 128-row block stores 516
frame cols (out cols plus the +-2 halo), so ONE frame tile per row
shift i serves all 5 column taps j of its group: the column shift is
folded into the host weight layout (w'[c'] = w[c'-j]) and the PE reads
each product tile at free-dim offset j when accumulating bank b over
cols [b*516+j, b*516+j+512).  Frame DMA drops from 10.6 to 5.3 MB/core.

Engine split:
  DVE   - 25 products w_t*f_t (fp16 2x mode, ~2.2us each) plus the
          t=0 weight dequant (rides the ramp shadow)
  ACT   - 15 weight dequants (int8 codes -> fp16 copy, 3.6us each)
  DMA   - 9 weight tiles stored as fp16 codes in DRAM and loaded
          directly (no dequant anywhere; costs +0.53 MB of DMA each,
          cheaper than an engine cast while the queues have slack;
          SWDGE casting DMA was measured to cost read+write on the
          queues, worse than both)
  PE    - accumulates the 25 product streams into PSUM fp32 via
          matmuls against a stationary (2^-5 * I); 8 banks = [128,512]
  tail  - PSUM evac split scalar/vector engines, output stored in two
          0.5 MB chunks

Group 0 is split into per-image half tiles ([128,2064]) so the first
dequant+mul start as soon as ~0.25 MB lands instead of waiting for the
full 1.6 MB head (the DMA queues round-robin all in-flight transfers,
so first-tile latency is proportional to bytes in flight).

Weights are int8 (w8 = clip(round(w * 32), -127, 127)); the 2^-5
dequant scale is folded into the PE's stationary identity, so every
dequant path (ACT copy, DVE copy, casting DMA) is a pure cast.

Host layouts:
  fin [5, 128, 4128] fp16: fin[i][p, (img,blk,c)] =
     Fpad[img, blk*128+p+i, c], Fpad = pad(F, rows 2/2, cols 2/2),
     c in [0,516).
  win [25, 128, 4128] int8: win[t][p, (img,blk,c')] =
     clip(round(32*core[img, t, blk*128+p, c'-j]), -127, 127) for
     c'-j in [0,512) else 0, where j = t%5.
  oout [128, 4096] fp16 (host casts to f32).
"""

import os
import sys

import numpy as np

for _p in ("/opt/trn_rl_repo",):
    if _p not in sys.path and os.path.isdir(_p):
        sys.path.insert(0, _p)

K = 5
NCORES = 8
IMGS_PER_CORE = 2
H = W = 512
NBLK = 4          # 128-row blocks per image
C_BLK = 516       # 512 out cols + 4 halo cols (-2..513)
FREE = IMGS_PER_CORE * NBLK * C_BLK   # 4128
HFREE = FREE // 2                     # 2064 = one image
O_FREE = IMGS_PER_CORE * NBLK * W     # 4096
NBANK = 8
BANK = 512
WSCALE = 2.0 ** -5  # int8 weight dequant scale, folded into PE identity

# Dequant engine assignment per tap t = 5*i + j.
DVE_DEQ = frozenset()
FP16_W = (0, 1, 4, 8, 9, 13, 14, 18, 19, 23, 24)  # stored fp16, no dequant
FP16_IDX = {t: n for n, t in enumerate(FP16_W)}
# remaining 15 taps dequant on ACT

_compiled = {}
last_results = None  # BassKernelResults of the most recent run (for test.py)


def _build_nc():
    import concourse.bacc as bacc
    import concourse.mybir as mybir
    from concourse.tile import TileContext

    f16 = mybir.dt.float16
    f32 = mybir.dt.float32
    i8 = mybir.dt.int8

    nc = bacc.Bacc(None, target_bir_lowering=False, debug=False)
    ident = nc.dram_tensor("ident", [128, 128], f16, kind="ExternalInput")
    fin = nc.dram_tensor("fin", [K, 128, FREE], f16, kind="ExternalInput")
    win = nc.dram_tensor("win", [K * K, 128, FREE], i8, kind="ExternalInput")
    win16 = nc.dram_tensor("win16", [len(FP16_W), 128, FREE], f16,
                           kind="ExternalInput")
    oout = nc.dram_tensor("oout", [128, O_FREE], f16, kind="ExternalOutput")

    n_streams = K * K

    with TileContext(nc) as tc:
        with (
            tc.tile_pool(name="ipool", bufs=1) as ipool,
            tc.tile_pool(name="fpool", bufs=3) as fpool,
            tc.tile_pool(name="fhpool", bufs=1) as fhpool,
            tc.tile_pool(name="w8pool", bufs=2) as w8pool,
            tc.tile_pool(name="whpool", bufs=1) as whpool,
            tc.tile_pool(name="wpool", bufs=2) as wpool,
            tc.tile_pool(name="spool", bufs=3) as spool,
            tc.tile_pool(name="shpool", bufs=3) as shpool,
            tc.tile_pool(name="opool", bufs=1) as opool,
            tc.tile_pool(name="ppool", bufs=1, space="PSUM") as ppool,
        ):
            id_t = ipool.tile([128, 128], f16, tag="ident")

            banks = [ppool.tile([128, BANK], f32, tag=f"b{b}",
                                name=f"bank{b}")
                     for b in range(NBANK)]
            osb = opool.tile([128, O_FREE], f16, tag="osb")

            f_tiles = {}
            w8_tiles = {}
            w_tiles = {}
            bank_n = [0] * NBANK

            def pe_acc(tile, j, bank_list, off0):
                # rhs covers out cols of bank b at product offset j
                for lb, b in enumerate(bank_list):
                    s = bank_n[b]
                    bank_n[b] += 1
                    nc.tensor.matmul(
                        out=banks[b][:],
                        lhsT=id_t[:],
                        rhs=tile[:][:, off0 + lb * C_BLK + j:
                                    off0 + lb * C_BLK + j + BANK],
                        start=(s == 0),
                        stop=(s == n_streams - 1),
                    )

            def emit_w(tg, k):
                t = tg * K + k
                if t in FP16_IDX:
                    # fp16 codes straight from DRAM, no dequant step
                    w_t = wpool.tile([128, FREE], f16, tag=f"w{k}",
                                     name=f"wf16_{t}")
                    nc.sync.dma_start(out=w_t[:], in_=win16[FP16_IDX[t]])
                    w_tiles[t] = w_t
                    return
                w8_t = w8pool.tile([128, FREE], i8, tag=f"w8{k}",
                                   name=f"w8_{t}")
                nc.sync.dma_start(out=w8_t[:], in_=win[t])
                w8_tiles[t] = w8_t
                # ACT dequants are emitted here so ACT chases the DMA
                # arrivals a group ahead of the DVE muls; DVE's own
                # dequants are emitted inline in emit_compute.
                if t not in DVE_DEQ:
                    w_t = wpool.tile([128, FREE], f16, tag=f"w{k}",
                                     name=f"wdq{t}")
                    nc.scalar.copy(out=w_t[:], in_=w8_t[:])
                    w_tiles[t] = w_t

            def emit_compute(tg):
                for k in range(K):
                    t = tg * K + k
                    if t in DVE_DEQ:
                        w_t = wpool.tile([128, FREE], f16, tag=f"w{k}",
                                         name=f"wdq{t}")
                        nc.vector.tensor_copy(out=w_t[:],
                                              in_=w8_tiles[t][:])
                        w_tiles[t] = w_t
                    if t == n_streams - 1:
                        # final tap in halves so the PSUM evac + store
                        # tail starts ~1us earlier
                        for h in range(2):
                            sl = slice(h * HFREE, (h + 1) * HFREE)
                            tmp = shpool.tile([128, HFREE], f16, tag="sh")
                            nc.vector.tensor_mul(
                                out=tmp[:], in0=w_tiles[t][:][:, sl],
                                in1=f_tiles[tg][:][:, sl])
                            pe_acc(tmp, k, range(4 * h, 4 * h + 4), 0)
                        continue
                    tmp = spool.tile([128, FREE], f16, tag="s")
                    nc.vector.tensor_mul(out=tmp[:], in0=w_tiles[t][:],
                                         in1=f_tiles[tg][:])
                    pe_acc(tmp, k, range(NBANK), 0)

            def emit_f(tg):
                f_t = fpool.tile([128, FREE], f16, tag="f",
                                 name=f"fr{tg}")
                nc.sync.dma_start(out=f_t[:], in_=fin[tg])
                f_tiles[tg] = f_t

            # ---- head: group 0 in quarter tiles (2 blocks each) so the
            # first mul starts as soon as ~0.5 MB lands; t0/t1 are
            # fp16-direct so the first muls are DMA-gated only ----
            QT = FREE // 4  # 1032 = 2 blocks
            wq = {}
            fq = []
            for q in range(4):
                fq.append(fhpool.tile([128, QT], f16, tag=f"fq{q}",
                                      name=f"fq{q}"))
            for q in range(4):
                wt = whpool.tile([128, QT], f16, tag=f"wq{q}",
                                 name=f"w0q{q}")
                nc.sync.dma_start(
                    out=wt[:], in_=win16[FP16_IDX[0]][:, q * QT:(q + 1) * QT])
                wq[(0, q)] = wt
                nc.sync.dma_start(
                    out=fq[q][:], in_=fin[0][:, q * QT:(q + 1) * QT])

            # DMAs globally ordered by consumption deadline: ACT-cast
            # (int8) taps get issued ~a group ahead of the fp16-direct
            # taps, since their arrival is followed by a 3.6us serial
            # cast on ACT before the DVE mul can consume them.
            emit_w(0, 2)
            emit_w(0, 3)
            for q in range(4):
                wt = whpool.tile([128, QT], f16, tag=f"wq{q}",
                                 name=f"w1q{q}")
                nc.sync.dma_start(
                    out=wt[:], in_=win16[FP16_IDX[1]][:, q * QT:(q + 1) * QT])
                wq[(1, q)] = wt
            emit_w(1, 0)
            emit_w(1, 1)
            emit_w(0, 4)
            emit_w(1, 2)
            emit_f(1)
            # identity rides the (empty) ACT hwdge ring, keeping its 128
            # tiny descriptors out of the sync ring's ramp window
            nc.scalar.dma_start(out=id_t[:], in_=ident[:])

            # group 0 compute: 20 quarter-muls, no casts anywhere
            for k in range(K):
                for q in range(4):
                    if k in (0, 1):
                        w_ap = wq[(k, q)][:]
                    else:
                        w_ap = w_tiles[k][:][:, q * QT:(q + 1) * QT]
                    tmp = shpool.tile([128, QT], f16, tag="sq")
                    nc.vector.tensor_mul(out=tmp[:], in0=w_ap,
                                         in1=fq[q][:])
                    pe_acc(tmp, k, [2 * q, 2 * q + 1], 0)

            emit_w(2, 0)
            emit_w(1, 3)
            emit_w(2, 1)
            emit_w(1, 4)
            emit_w(2, 2)
            emit_f(2)
            emit_compute(1)
            emit_w(3, 0)
            emit_w(2, 3)
            emit_w(3, 1)
            emit_w(2, 4)
            emit_w(3, 2)
            emit_f(3)
            emit_compute(2)
            emit_w(4, 0)
            emit_w(3, 3)
            emit_w(4, 1)
            emit_w(3, 4)
            emit_w(4, 2)
            emit_f(4)
            emit_compute(3)
            emit_w(4, 3)
            emit_w(4, 4)
            emit_compute(K - 1)

            # ---- tail: PSUM fp32 -> SBUF fp16 split across scalar and
            # vector engines, store output in two chunks ----
            for b in range(NBANK):
                dst = osb[:][:, b * BANK:(b + 1) * BANK]
                if b % 2 == 0:
                    nc.scalar.copy(out=dst, in_=banks[b][:])
                else:
                    nc.vector.tensor_copy(out=dst, in_=banks[b][:])
                nc.sync.dma_start(out=oout[:, b * BANK:(b + 1) * BANK],
                                  in_=osb[:][:, b * BANK:(b + 1) * BANK])
    nc.finalize()
    return nc


def _host_prep(frames, core):
    """Build per-core in_maps. frames [4,4,1,512,512] f32, core [4,4,25,1,512,512]."""
    G = NCORES * IMGS_PER_CORE  # 16
    F = np.ascontiguousarray(frames.reshape(G, H, W))
    Wc = core.reshape(G, K * K, H, W)

    # frames: pad rows 2/2, cols 2/2 -> [G, 516, 516]
    Fp = np.pad(F, ((0, 0), (2, 2), (2, 2))).astype(np.float16)
    # A[g, i, blk, p, c] = Fp[g, blk*128+p+i, c]
    A = np.empty((G, K, NBLK, 128, C_BLK), np.float16)
    for i in range(K):
        A[:, i] = Fp[:, i:i + H, :].reshape(G, NBLK, 128, C_BLK)
    # fin[core][i, p, (img, blk, c)]
    fprep = np.ascontiguousarray(
        A.reshape(NCORES, IMGS_PER_CORE, K, NBLK, 128, C_BLK)
        .transpose(0, 2, 4, 1, 3, 5))

    # weights: codes of w/2^-5, column-shifted by j so products line up
    # with an aligned frame read; PE reads the product at offset j.
    # int8-rounded codes for the engine-cast taps, full fp16 codes for
    # the direct-load taps.
    w8 = np.clip(np.round(Wc * (1.0 / WSCALE)), -127, 127).astype(np.int8)
    Ws = np.zeros((G, K * K, H, C_BLK), np.int8)
    for j in range(K):
        Ws[:, j::K, :, j:j + W] = w8[:, j::K]
    wprep = np.ascontiguousarray(
        Ws.reshape(NCORES, IMGS_PER_CORE, K * K, NBLK, 128, C_BLK)
        .transpose(0, 2, 4, 1, 3, 5))

    tf = list(FP16_W)
    Wf = np.zeros((G, len(tf), H, C_BLK), np.float16)
    for n, t in enumerate(tf):
        j = t % K
        Wf[:, n, :, j:j + W] = (Wc[:, t] * (1.0 / WSCALE)).astype(np.float16)
    wfprep = np.ascontiguousarray(
        Wf.reshape(NCORES, IMGS_PER_CORE, len(tf), NBLK, 128, C_BLK)
        .transpose(0, 2, 4, 1, 3, 5))

    ident = (np.eye(128) * WSCALE).astype(np.float16)
    in_maps = []
    for c in range(NCORES):
        in_maps.append({
            "ident": ident,
            "fin": fprep[c].reshape(K, 128, FREE),
            "win": wprep[c].reshape(K * K, 128, FREE),
            "win16": wfprep[c].reshape(len(tf), 128, FREE),
        })
    return in_maps


def kernel(frames, core, bias):
    global last_results
    from concourse.bass_utils import run_bass_kernel_spmd

    frames = np.asarray(frames, dtype=np.float32)
    core = np.asarray(core, dtype=np.float32)

    if "nc" not in _compiled:
        _compiled["nc"] = _build_nc()
    nc = _compiled["nc"]

    in_maps = _host_prep(frames, core)
    trace = os.environ.get("KC_TRACE") == "1"
    tmpdir = os.environ.get("KC_TRACE_DIR") or None
    if tmpdir:
        os.makedirs(tmpdir, exist_ok=True)
    res = run_bass_kernel_spmd(nc, in_maps, list(range(NCORES)), trace=trace,
                               tmpdir=tmpdir)
    last_results = res

    G = NCORES * IMGS_PER_CORE
    out = np.empty((G, H, W), np.float32)
    for c in range(NCORES):
        o = res.results[c]["oout"]  # [128, 4096] f16
        ov = o.reshape(128, IMGS_PER_CORE, NBLK, W).astype(np.float32)
        for img in range(IMGS_PER_CORE):
            out[c * IMGS_PER_CORE + img] = (
                ov[:, img].transpose(1, 0, 2).reshape(H, W))
    return out.reshape(4, 4, H, W)


# revision 21
# speedup vs baseline: 1.0990x; 1.0282x over previous
"""Per-pixel adaptive 5x5 conv (KPN) for Trainium2, 8-core data parallel.

out[g,h,w] = sum_{i,j} core[g,5i+j,h,w] * frames_pad[g,h+i-2,w+j-2]
with g = flattened (B,N) = 16 image planes; 2 planes per NeuronCore,
fused into one free dim so every elementwise op covers both.

v2 layout (vs v1's parity-copy scheme): each 128-row block stores 516
frame cols (out cols plus the +-2 halo), so ONE frame tile per row
shift i serves all 5 column taps j of its group: the column shift is
folded into the host weight layout (w'[c'] = w[c'-j]) and the PE reads
each product tile at free-dim offset j when accumulating bank b over
cols [b*516+j, b*516+j+512).  Frame DMA drops from 10.6 to 5.3 MB/core.

Engine split:
  DVE   - 25 products w_t*f_t (fp16 2x mode, ~2.2us each) plus the
          t=0 weight dequant (rides the ramp shadow)
  ACT   - 15 weight dequants (int8 codes -> fp16 copy, 3.6us each)
  DMA   - 9 weight tiles stored as fp16 codes in DRAM and loaded
          directly (no dequant anywhere; costs +0.53 MB of DMA each,
          cheaper than an engine cast while the queues have slack;
          SWDGE casting DMA was measured to cost read+write on the
          queues, worse than both)
  PE    - accumulates the 25 product streams into PSUM fp32 via
          matmuls against a stationary (2^-5 * I); 8 banks = [128,512]
  tail  - PSUM evac split scalar/vector engines, output stored in two
          0.5 MB chunks

Group 0 is split into per-image half tiles ([128,2064]) so the first
dequant+mul start as soon as ~0.25 MB lands instead of waiting for the
full 1.6 MB head (the DMA queues round-robin all in-flight transfers,
so first-tile latency is proportional to bytes in flight).

Weights are int8 (w8 = clip(round(w * 32), -127, 127)); the 2^-5
dequant scale is folded into the PE's stationary identity, so every
dequant path (ACT copy, DVE copy, casting DMA) is a pure cast.

Host layouts:
  fin [5, 128, 4128] fp16: fin[i][p, (img,blk,c)] =
     Fpad[img, blk*128+p+i, c], Fpad = pad(F, rows 2/2, cols 2/2),
     c in [0,516).
  win [25, 128, 4128] int8: win[t][p, (img,blk,c')] =
     clip(round(32*core[img, t, blk*128+p, c'-j]), -127, 127) for
     c'-j in [0,512) else 0, where j = t%5.
  oout [128, 4096] fp16 (host casts to f32).
"""

import os
import sys

import numpy as np

for _p in ("/opt/trn_rl_repo",):
    if _p not in sys.path and os.path.isdir(_p):
        sys.path.insert(0, _p)

K = 5
NCORES = 8
IMGS_PER_CORE = 2
H = W = 512
NBLK = 4          # 128-row blocks per image
C_BLK = 516       # 512 out cols + 4 halo cols (-2..513)
FREE = IMGS_PER_CORE * NBLK * C_BLK   # 4128
HFREE = FREE // 2                     # 2064 = one image
O_FREE = IMGS_PER_CORE * NBLK * W     # 4096
NBANK = 8
BANK = 512
WSCALE = 2.0 ** -5  # int8 weight dequant scale, folded into PE identity

# Dequant engine assignment per tap t = 5*i + j.
DVE_DEQ = frozenset()
FP16_W = (0, 1, 4, 8, 9, 13, 14, 18, 19, 23, 24)  # stored fp16, no dequant
FP16_IDX = {t: n for n, t in enumerate(FP16_W)}
# remaining 15 taps dequant on ACT

_compiled = {}
last_results = None  # BassKernelResults of the most recent run (for test.py)


def _build_nc():
    import concourse.bacc as bacc
    import concourse.mybir as mybir
    from concourse.tile import TileContext

    f16 = mybir.dt.float16
    f32 = mybir.dt.float32
    i8 = mybir.dt.int8

    nc = bacc.Bacc(None, target_bir_lowering=False, debug=False)
    ident = nc.dram_tensor("ident", [128, 128], f16, kind="ExternalInput")
    fin = nc.dram_tensor("fin", [K, 128, FREE], f16, kind="ExternalInput")
    win = nc.dram_tensor("win", [K * K, 128, FREE], i8, kind="ExternalInput")
    win16 = nc.dram_tensor("win16", [len(FP16_W), 128, FREE], f16,
                           kind="ExternalInput")
    oout = nc.dram_tensor("oout", [128, O_FREE], f16, kind="ExternalOutput")

    n_streams = K * K

    with TileContext(nc) as tc:
        with (
            tc.tile_pool(name="ipool", bufs=1) as ipool,
            tc.tile_pool(name="fpool", bufs=3) as fpool,
            tc.tile_pool(name="fhpool", bufs=1) as fhpool,
            tc.tile_pool(name="w8pool", bufs=2) as w8pool,
            tc.tile_pool(name="whpool", bufs=1) as whpool,
            tc.tile_pool(name="wpool", bufs=2) as wpool,
            tc.tile_pool(name="spool", bufs=3) as spool,
            tc.tile_pool(name="shpool", bufs=3) as shpool,
            tc.tile_pool(name="opool", bufs=1) as opool,
            tc.tile_pool(name="ppool", bufs=1, space="PSUM") as ppool,
        ):
            id_t = ipool.tile([128, 128], f16, tag="ident")

            banks = [ppool.tile([128, BANK], f32, tag=f"b{b}",
                                name=f"bank{b}")
                     for b in range(NBANK)]
            osb = opool.tile([128, O_FREE], f16, tag="osb")

            f_tiles = {}
            w8_tiles = {}
            w_tiles = {}
            bank_n = [0] * NBANK

            def pe_acc(tile, j, bank_list, off0):
                # rhs covers out cols of bank b at product offset j
                for lb, b in enumerate(bank_list):
                    s = bank_n[b]
                    bank_n[b] += 1
                    nc.tensor.matmul(
                        out=banks[b][:],
                        lhsT=id_t[:],
                        rhs=tile[:][:, off0 + lb * C_BLK + j:
                                    off0 + lb * C_BLK + j + BANK],
                        start=(s == 0),
                        stop=(s == n_streams - 1),
                    )

            def emit_w(tg, k):
                t = tg * K + k
                if t in FP16_IDX:
                    # fp16 codes straight from DRAM, no dequant step
                    w_t = wpool.tile([128, FREE], f16, tag=f"w{k}",
                                     name=f"wf16_{t}")
                    nc.sync.dma_start(out=w_t[:], in_=win16[FP16_IDX[t]])
                    w_tiles[t] = w_t
                    return
                w8_t = w8pool.tile([128, FREE], i8, tag=f"w8{k}",
                                   name=f"w8_{t}")
                nc.sync.dma_start(out=w8_t[:], in_=win[t])
                w8_tiles[t] = w8_t
                # ACT dequants are emitted here so ACT chases the DMA
                # arrivals a group ahead of the DVE muls; DVE's own
                # dequants are emitted inline in emit_compute.
                if t not in DVE_DEQ:
                    w_t = wpool.tile([128, FREE], f16, tag=f"w{k}",
                                     name=f"wdq{t}")
                    nc.scalar.copy(out=w_t[:], in_=w8_t[:])
                    w_tiles[t] = w_t

            def emit_compute(tg):
                for k in range(K):
                    t = tg * K + k
                    if t in DVE_DEQ:
                        w_t = wpool.tile([128, FREE], f16, tag=f"w{k}",
                                         name=f"wdq{t}")
                        nc.vector.tensor_copy(out=w_t[:],
                                              in_=w8_tiles[t][:])
                        w_tiles[t] = w_t
                    tmp = spool.tile([128, FREE], f16, tag="s")
                    nc.vector.tensor_mul(out=tmp[:], in0=w_tiles[t][:],
                                         in1=f_tiles[tg][:])
                    pe_acc(tmp, k, range(NBANK), 0)

            def emit_compute_last(tg):
                # last group in per-image half passes: banks 0-3 receive
                # their final accumulation one pass early, so their PSUM
                # evac (on ACT, which is idle by then) and the first
                # output store overlap the second half's muls.
                for h in range(2):
                    for k in range(K):
                        t = tg * K + k
                        sl = slice(h * HFREE, (h + 1) * HFREE)
                        tmp = shpool.tile([128, HFREE], f16, tag="sh")
                        nc.vector.tensor_mul(
                            out=tmp[:], in0=w_tiles[t][:][:, sl],
                            in1=f_tiles[tg][:][:, sl])
                        pe_acc(tmp, k, range(4 * h, 4 * h + 4), 0)
                    if h == 0:
                        for b in range(4):
                            nc.scalar.copy(
                                out=osb[:][:, b * BANK:(b + 1) * BANK],
                                in_=banks[b][:])
                        nc.sync.dma_start(out=oout[:, :O_FREE // 2],
                                          in_=osb[:][:, :O_FREE // 2])

            def emit_f(tg):
                f_t = fpool.tile([128, FREE], f16, tag="f",
                                 name=f"fr{tg}")
                nc.sync.dma_start(out=f_t[:], in_=fin[tg])
                f_tiles[tg] = f_t

            # ---- head: group 0 in quarter tiles (2 blocks each) so the
            # first mul starts as soon as ~0.5 MB lands; t0/t1 are
            # fp16-direct so the first muls are DMA-gated only ----
            QT = FREE // 4  # 1032 = 2 blocks
            wq = {}
            fq = []
            for q in range(4):
                fq.append(fhpool.tile([128, QT], f16, tag=f"fq{q}",
                                      name=f"fq{q}"))
            for q in range(4):
                wt = whpool.tile([128, QT], f16, tag=f"wq{q}",
                                 name=f"w0q{q}")
                nc.sync.dma_start(
                    out=wt[:], in_=win16[FP16_IDX[0]][:, q * QT:(q + 1) * QT])
                wq[(0, q)] = wt
                nc.sync.dma_start(
                    out=fq[q][:], in_=fin[0][:, q * QT:(q + 1) * QT])

            # DMAs globally ordered by consumption deadline: ACT-cast
            # (int8) taps get issued ~a group ahead of the fp16-direct
            # taps, since their arrival is followed by a 3.6us serial
            # cast on ACT before the DVE mul can consume them.
            emit_w(0, 2)
            emit_w(0, 3)
            for q in range(4):
                wt = whpool.tile([128, QT], f16, tag=f"wq{q}",
                                 name=f"w1q{q}")
                nc.sync.dma_start(
                    out=wt[:], in_=win16[FP16_IDX[1]][:, q * QT:(q + 1) * QT])
                wq[(1, q)] = wt
            emit_w(1, 0)
            emit_w(1, 1)
            emit_w(0, 4)
            emit_w(1, 2)
            emit_f(1)
            # identity rides the (empty) ACT hwdge ring, keeping its 128
            # tiny descriptors out of the sync ring's ramp window
            nc.scalar.dma_start(out=id_t[:], in_=ident[:])

            # group 0 compute: 20 quarter-muls, no casts anywhere
            for k in range(K):
                for q in range(4):
                    if k in (0, 1):
                        w_ap = wq[(k, q)][:]
                    else:
                        w_ap = w_tiles[k][:][:, q * QT:(q + 1) * QT]
                    tmp = shpool.tile([128, QT], f16, tag="sq")
                    nc.vector.tensor_mul(out=tmp[:], in0=w_ap,
                                         in1=fq[q][:])
                    pe_acc(tmp, k, [2 * q, 2 * q + 1], 0)

            emit_w(2, 0)
            emit_w(1, 3)
            emit_w(2, 1)
            emit_w(1, 4)
            emit_w(2, 2)
            emit_f(2)
            emit_compute(1)
            emit_w(3, 0)
            emit_w(2, 3)
            emit_w(3, 1)
            emit_w(2, 4)
            emit_w(3, 2)
            emit_f(3)
            emit_compute(2)
            emit_w(4, 0)
            emit_w(3, 3)
            emit_w(4, 1)
            emit_w(3, 4)
            emit_w(4, 2)
            emit_f(4)
            emit_compute(3)
            emit_w(4, 3)
            emit_w(4, 4)
            emit_compute_last(K - 1)

            # ---- tail: banks 4-7 PSUM -> SBUF fp16 split across scalar
            # and vector engines, then the second output chunk ----
            for b in range(4, NBANK):
                dst = osb[:][:, b * BANK:(b + 1) * BANK]
                if b % 2 == 0:
                    nc.scalar.copy(out=dst, in_=banks[b][:])
                else:
                    nc.vector.tensor_copy(out=dst, in_=banks[b][:])
            nc.sync.dma_start(out=oout[:, O_FREE // 2:],
                              in_=osb[:][:, O_FREE // 2:])
    nc.finalize()
    return nc


def _host_prep(frames, core):
    """Build per-core in_maps. frames [4,4,1,512,512] f32, core [4,4,25,1,512,512]."""
    G = NCORES * IMGS_PER_CORE  # 16
    F = np.ascontiguousarray(frames.reshape(G, H, W))
    Wc = core.reshape(G, K * K, H, W)

    # frames: pad rows 2/2, cols 2/2 -> [G, 516, 516]
    Fp = np.pad(F, ((0, 0), (2, 2), (2, 2))).astype(np.float16)
    # A[g, i, blk, p, c] = Fp[g, blk*128+p+i, c]
    A = np.empty((G, K, NBLK, 128, C_BLK), np.float16)
    for i in range(K):
        A[:, i] = Fp[:, i:i + H, :].reshape(G, NBLK, 128, C_BLK)
    # fin[core][i, p, (img, blk, c)]
    fprep = np.ascontiguousarray(
        A.reshape(NCORES, IMGS_PER_CORE, K, NBLK, 128, C_BLK)
        .transpose(0, 2, 4, 1, 3, 5))

    # weights: codes of w/2^-5, column-shifted by j so products line up
    # with an aligned frame read; PE reads the product at offset j.
    # int8-rounded codes for the engine-cast taps, full fp16 codes for
    # the direct-load taps.
    w8 = np.clip(np.round(Wc * (1.0 / WSCALE)), -127, 127).astype(np.int8)
    Ws = np.zeros((G, K * K, H, C_BLK), np.int8)
    for j in range(K):
        Ws[:, j::K, :, j:j + W] = w8[:, j::K]
    wprep = np.ascontiguousarray(
        Ws.reshape(NCORES, IMGS_PER_CORE, K * K, NBLK, 128, C_BLK)
        .transpose(0, 2, 4, 1, 3, 5))

    tf = list(FP16_W)
    Wf = np.zeros((G, len(tf), H, C_BLK), np.float16)
    for n, t in enumerate(tf):
        j = t % K
        Wf[:, n, :, j:j + W] = (Wc[:, t] * (1.0 / WSCALE)).astype(np.float16)
    wfprep = np.ascontiguousarray(
        Wf.reshape(NCORES, IMGS_PER_CORE, len(tf), NBLK, 128, C_BLK)
        .transpose(0, 2, 4, 1, 3, 5))

    ident = (np.eye(128) * WSCALE).astype(np.float16)
    in_maps = []
    for c in range(NCORES):
        in_maps.append({
            "ident": ident,
            "fin": fprep[c].reshape(K, 128, FREE),
            "win": wprep[c].reshape(K * K, 128, FREE),
            "win16": wfprep[c].reshape(len(tf), 128, FREE),
        })
    return in_maps


def kernel(frames, core, bias):
    global last_results
    from concourse.bass_utils import run_bass_kernel_spmd

    frames = np.asarray(frames, dtype=np.float32)
    core = np.asarray(core, dtype=np.float32)

    if "nc" not in _compiled:
        _compiled["nc"] = _build_nc()
    nc = _compiled["nc"]

    in_maps = _host_prep(frames, core)
    trace = os.environ.get("KC_TRACE") == "1"
    tmpdir = os.environ.get("KC_TRACE_DIR") or None
    if tmpdir:
        os.makedirs(tmpdir, exist_ok=True)
    res = run_bass_kernel_spmd(nc, in_maps, list(range(NCORES)), trace=trace,
                               tmpdir=tmpdir)
    last_results = res

    G = NCORES * IMGS_PER_CORE
    out = np.empty((G, H, W), np.float32)
    for c in range(NCORES):
        o = res.results[c]["oout"]  # [128, 4096] f16
        ov = o.reshape(128, IMGS_PER_CORE, NBLK, W).astype(np.float32)
        for img in range(IMGS_PER_CORE):
            out[c * IMGS_PER_CORE + img] = (
                ov[:, img].transpose(1, 0, 2).reshape(H, W))
    return out.reshape(4, 4, H, W)
